# revision 64
# baseline (speedup 1.0000x reference)
"""Trainium2 Bass kernel for nn_Attention (AdderNet attention block).

Problem: B=8, S=197, E=384, H=6, D=64.
  x2d = x.reshape(E, B*S)                      # flat reshape, [384, 1576]
  per proj (q,k,v):  Y = -sum_ci |x2d[ci,n] - w[co,ci]|   (adder 1x1)
                     LN over ALL of [E,B,S] (elementwise affine params)
                     flat-reshape to [B,S,H,D] -> heads
  att = softmax(q k^T * scale) + I; o = att v; token-LN; fc.

Sharding: core c owns co-rows [48c, 48c+48) of each of the three adder
projections; those rows are exactly the post-LN data needed for batch
b=c of the attention, so attention + out-LN + fc are fully local per
core.  Cross-core exchange: two tiny AllReduces of LN statistics
(q+k stats early, v stats at phase-A end) overlapped with compute.

Adder projection via separable decomposition (instead of dense
elementwise abs-diff):
  |x-w| = |x| - sign(x)*w + relu(|w|-|x|)*(1 + sign(x)sign(w))
and relu(u-t) ~= a0(u) + sum_k ak(u) * min(t,tau_k)   (LS fit over
t ~ |N(0,1)|), giving
  Y = (W - Sw*A0)@s - sum_k AEk@m_k - sum_k (Sw*AOk)@c_k
      - 1@(axb+rb) - c0[co]
with device basis tiles s=sign(x), m_k=min(|x|,tau_k), c_k=s*m_k,
axb=bf16(|x|), rb=|x|-axb (residual, so colsum is fp32-accurate) in
bf16; weight tables host-precomputed in bf16; all 216 reductions are
PE matmuls.  Stats (sum, sum-of-squares) accumulate during PSUM
evacuation.  End-to-end rel err ~1.8e-3 (verified vs exact reference).
"""

import numpy as np
from contextlib import ExitStack

B, S, E = 8, 197, 384
H, D = 6, 64
N = B * S            # 1576
RPC = E // 8         # 48 rows per core per projection
NCORE = 8
NTOT = E * N         # 605184 elements per projection
C_SHIFT = 307.0      # conditioning shift for sum-of-squares (Y ~ -307)
EPS = 1e-5
SCALE = float((2.0 * D * (1.0 - 2.0 / np.pi)) ** (-0.5))
NCH = [(0, 512), (512, 1024), (1024, 1536), (1536, 1576)]
SBLK = [(0, 128), (128, 197)]     # token blocks of 197
EBLK = [(0, 128), (128, 256), (256, 384)]

TAUS = (0.05, 0.11, 0.2)
K = len(TAUS)
NB = 2 * K + 2       # basis: s, m_1..K, c_1..K, axb
CO = 3 * RPC         # 144 rows of stacked q/k/v weights per core
COG = [(0, 96), (96, 144)]        # co-groups: qk (M=96), v (M=48)

_PROGRAM = None


def _build_program(no_collective=False):
    import concourse.bass as bass
    import concourse.mybir as mybir
    from concourse import bacc, tile

    dt = mybir.dt
    f32 = dt.float32
    bf16 = dt.bfloat16
    AF = mybir.ActivationFunctionType
    OP = mybir.AluOpType

    nc = bacc.Bacc(num_devices=NCORE)

    # ---- I/O ----
    x2d_d = nc.dram_tensor("x2d", [E, N], bf16, kind="ExternalInput")
    wbt_d = nc.dram_tensor("wbt", [128, NB * 3 * CO], bf16,
                           kind="ExternalInput")
    negc0_d = nc.dram_tensor("negc0", [128, 4], f32, kind="ExternalInput")
    onesrow_d = nc.dram_tensor("onesrow", [1, 128], f32, kind="ExternalInput")
    onescol_d = nc.dram_tensor("onescol", [128, 2], f32, kind="ExternalInput")
    indqk_d = nc.dram_tensor("indqk", [128, 2], f32, kind="ExternalInput")
    lnwT_d = nc.dram_tensor("dlnwT", [2, E, S], bf16, kind="ExternalInput")
    lnbT_d = nc.dram_tensor("lnbT", [2, E, S], bf16, kind="ExternalInput")
    lnwv_d = nc.dram_tensor("dlnwv", [S, E], bf16, kind="ExternalInput")
    lnbv_d = nc.dram_tensor("lnbv", [S, E], bf16, kind="ExternalInput")
    fcwt_d = nc.dram_tensor("fcwt", [E, E], f32, kind="ExternalInput")
    fcb_d = nc.dram_tensor("fcb1", [1, E], f32, kind="ExternalInput")
    eyeq_d = nc.dram_tensor("eyeq", [128, 394], f32, kind="ExternalInput")
    cst_d = nc.dram_tensor("cstcol", [128, 2], f32, kind="ExternalInput")
    out_d = nc.dram_tensor("out", [S, E], f32, kind="ExternalOutput")

    # internal DRAM
    ybuf = [nc.dram_tensor(f"ybuf{p}", [RPC * N], f32) for p in range(3)]

    with ExitStack() as ctx:
        tc = ctx.enter_context(tile.TileContext(nc))
        const = ctx.enter_context(tc.tile_pool(name="const", bufs=1))

        # x tiles FIRST: everything downstream gates on them.  wbt arrives
        # in per-basis slices so the first matmul stack can start early.
        xp0 = ctx.enter_context(tc.tile_pool(name="xp0", bufs=1))
        xts = []
        wbt = const.tile([128, NB * 3 * CO], bf16)
        for t in range(3):
            xt = xp0.tile([128, N], bf16, name=f"xt{t}")
            nc.sync.dma_start(xt[:], x2d_d[128 * t:128 * t + 128, :])
            xts.append(xt)
            if t == 0:
                nc.sync.dma_start(wbt[:, 0:3 * CO], wbt_d[:, 0:3 * CO])
        for b in range(1, NB):
            nc.sync.dma_start(wbt[:, b * 3 * CO:(b + 1) * 3 * CO],
                              wbt_d[:, b * 3 * CO:(b + 1) * 3 * CO])
        negc0 = const.tile([128, 4], f32)
        nc.sync.dma_start(negc0[:], negc0_d[:])
        onesrow = const.tile([1, 128], f32)
        nc.sync.dma_start(onesrow[:], onesrow_d[:])
        onescol = const.tile([128, 2], f32)
        nc.sync.dma_start(onescol[:], onescol_d[:])
        indqk = const.tile([128, 2], f32)
        nc.sync.dma_start(indqk[:], indqk_d[:])
        cst = const.tile([128, 2], f32)
        nc.sync.dma_start(cst[:], cst_d[:])
        lnwT = {}
        lnbT = {}
        for p in range(2):
            for ei, (e0, e1) in enumerate(EBLK):
                dl = const.tile([128, S], bf16, name=f"dlnwT{p}{ei}")
                nc.sync.dma_start(dl[:], lnwT_d[p, e0:e1, :])
                lw = const.tile([128, S], f32, name=f"lnwT{p}{ei}")
                nc.vector.tensor_scalar(lw[:], dl[:], 1.0, None, OP.add)
                lnwT[(p, ei)] = lw
                lb = const.tile([128, S], bf16, name=f"lnbT{p}{ei}")
                nc.sync.dma_start(lb[:], lnbT_d[p, e0:e1, :])
                lnbT[(p, ei)] = lb
        eyeq = const.tile([128, 394], f32)
        nc.sync.dma_start(eyeq[:], eyeq_d[:])
        fcb1 = const.tile([1, E], f32)
        nc.sync.dma_start(fcb1[:], fcb_d[:])
        fcw = []
        for ei, (e0, e1) in enumerate(EBLK):
            fw = const.tile([128, E], f32, tag=f"fw{ei}", name=f"fw{ei}")
            nc.sync.dma_start(fw[:], fcwt_d[e0:e1, :])
            fcw.append(fw)

        # stat result tiles
        rsv = const.tile([128, 3], f32)      # 1/sqrt(var+eps) per proj
        negmu = const.tile([128, 3], f32)    # -mu per proj
        negmurs = const.tile([128, 3], f32)  # -mu*rs per proj

        ccdram = ctx.enter_context(
            tc.tile_pool(name="ccdram", bufs=1, space="DRAM"))
        cc1_in = ccdram.tile([1, 24], f32, name="cc1_in")
        cc1_out = ccdram.tile([1, 24], f32, name="cc1_out")

        # warm the ACT table that holds Sqrt so the stat chain doesn't
        # pay a table load on the critical path.
        dum = const.tile([1, 4], f32)
        nc.vector.memset(dum[:], 1.0)
        warm = const.tile([1, 4], f32)
        nc.scalar.activation(warm[:], dum[:], AF.Sqrt)

        # ================= Phase A: separable adder projections ==========
        apool = ctx.enter_context(tc.tile_pool(name="apool", bufs=1))
        mctx = ExitStack()   # closed before phase B to release PSUM banks
        psT = mctx.enter_context(tc.tile_pool(name="psT", bufs=1,
                                              space="PSUM"))
        with ExitStack() as actx:
            bp = actx.enter_context(tc.tile_pool(name="bp", bufs=1))
            evp = actx.enter_context(tc.tile_pool(name="evp", bufs=3))

            # basis tiles bs[b][t]; b: 0=s, 1..K=m_k, K+1..2K=c_k, 2K+1=axb
            bs = [[bp.tile([128, N], bf16, name=f"bs{b}_{t}")
                   for t in range(3)] for b in range(NB)]

            for t in range(3):
                xt = xts[t]
                nc.scalar.activation(bs[0][t][:], xt[:], AF.Sign)
                nc.scalar.activation(bs[2 * K + 1][t][:], xt[:], AF.Abs)
                for k in range(K):
                    nc.vector.tensor_scalar(
                        bs[1 + k][t][:], bs[2 * K + 1][t][:], 1.0, TAUS[k],
                        OP.mult, OP.min)
                    nc.vector.tensor_tensor(
                        bs[1 + K + k][t][:], bs[1 + k][t][:], bs[0][t][:],
                        OP.mult)

            # stats accumulators (per-partition, per chunk: sum, sumsq)
            ssqk = apool.tile([128, 8], f32)
            ssv = apool.tile([128, 8], f32)

            # main matmul stacks, qk group first then v
            psQK = actx.enter_context(
                tc.tile_pool(name="psQK", bufs=3, space="PSUM"))
            psV = actx.enter_context(
                tc.tile_pool(name="psV", bufs=2, space="PSUM"))
            for gi, (co0, co1) in enumerate(COG):
                M = co1 - co0
                pool_g = psQK if gi == 0 else psV
                for ci_, (a, b_) in enumerate(NCH):
                    ps = pool_g.tile([M, b_ - a], f32, tag="ps")
                    i = 0
                    for b in range(NB):
                        for t in range(3):
                            off = (b * 3 + t) * CO + co0
                            nc.tensor.matmul(
                                ps[:], wbt[:, off:off + M],
                                bs[b][t][:, a:b_],
                                start=(i == 0), stop=(i == NB * 3 - 1))
                            i += 1
                    # evac + stats on DVE: ev = ps + negc0 (accum sum);
                    # t3 = ps + negc0 + C ; sq = t3*t3 (accum sumsq)
                    ev = evp.tile([M, b_ - a], f32, tag="evac")
                    ss = ssqk if gi == 0 else ssv
                    nc.vector.tensor_scalar(
                        ev[:], ps[:], negc0[0:M, gi:gi + 1], None, OP.add,
                        OP.add, accum_out=ss[0:M, 2 * ci_:2 * ci_ + 1])
                    t3 = evp.tile([M, b_ - a], f32, tag="t3ev")
                    nc.vector.tensor_scalar(
                        t3[:], ps[:], negc0[0:M, 2 + gi:3 + gi], None,
                        OP.add)
                    junk = evp.tile([M, b_ - a], f32, tag="junkev")
                    nc.vector.scalar_tensor_tensor(
                        junk[:], t3[:], 1.0, t3[:], OP.mult, OP.mult,
                        accum_out=ss[0:M, 2 * ci_ + 1:2 * ci_ + 2])
                    if gi == 0:
                        for p in range(2):
                            nc.gpsimd.dma_start(
                                ybuf[p][:].rearrange(
                                    "(r n) -> r n", n=N)[:, a:b_],
                                ev[48 * p:48 * p + 48, :])
                    else:
                        nc.gpsimd.dma_start(
                            ybuf[2][:].rearrange(
                                "(r n) -> r n", n=N)[:, a:b_],
                            ev[0:48, :])

                # per-group stats partition-reduction (PE + copy)
                if gi == 0:
                    prqk = psT.tile([2, 8], f32, tag="pr")
                    nc.tensor.matmul(prqk[:], indqk[0:96, :], ssqk[0:96, :],
                                     start=True, stop=True)
                    prqk_sb = apool.tile([2, 8], f32)
                    nc.scalar.copy(prqk_sb[:], prqk[:])
                    nc.gpsimd.dma_start(cc1_in[0:1, 0:8], prqk_sb[0:1, :])
                    nc.gpsimd.dma_start(cc1_in[0:1, 8:16], prqk_sb[1:2, :])
                else:
                    prv = psT.tile([1, 8], f32, tag="pr")
                    nc.tensor.matmul(prv[:], onescol[0:48, 0:1], ssv[0:48, :],
                                     start=True, stop=True)
                    prv_sb = apool.tile([1, 8], f32)
                    nc.scalar.copy(prv_sb[:], prv[:])
                    nc.gpsimd.dma_start(cc1_in[0:1, 16:24], prv_sb[:])
                    if no_collective:
                        nc.gpsimd.dma_start(cc1_out[:], cc1_in[:])
                    else:
                        nc.gpsimd.collective_compute(
                            "AllReduce", mybir.AluOpType.add,
                            replica_groups=[list(range(NCORE))],
                            ins=[cc1_in.opt()], outs=[cc1_out.opt()])

        # ============ pre-LN feature-major transposes for q, k ============
        # (overlaps the v matmul stacks and the q/k AllReduce)
        # G = YT * lnwT precomputed so post-AllReduce LN is 2 ops per tile.
        G = {}
        YT = {}
        ytp = mctx.enter_context(tc.tile_pool(name="ytp", bufs=4))
        for p in range(2):
            for ei, (e0, e1) in enumerate(EBLK):
                pst = psT.tile([128, S], f32, tag="pst")
                for si, (s0, s1) in enumerate(SBLK):
                    sP = s1 - s0
                    yt = ytp.tile([sP, 128], f32, tag="ytqk")
                    nc.sync.dma_start(
                        yt[:],
                        ybuf[p][:].rearrange(
                            "(s e) -> s e", e=E)[s0:s1, e0:e1])
                    nc.tensor.transpose(
                        pst[:, s0:s1], yt[:], eyeq[0:sP, 0:sP])
                ytt = apool.tile([128, S], f32, name=f"YT{p}{ei}")
                nc.scalar.copy(ytt[:], pst[:])
                YT[(p, ei)] = ytt
                g_ = apool.tile([128, S], f32, name=f"G{p}{ei}")
                nc.vector.tensor_tensor(g_[:], ytt[:], lnwT[(p, ei)][:],
                                        OP.mult)
                G[(p, ei)] = g_

        # ================= stats scalar math =================
        stq = mctx.enter_context(tc.tile_pool(name="stq", bufs=2))

        def stat_math(statb, pcol, ncols, out_idx):
            """statb: [128, ncols*8] bcast stats; for each local proj i,
            columns i*8 + (chunk*2 + j); compute rs, negmu into
            rsv/negmu[:, out_idx+i]."""
            for i in range(ncols):
                s1 = stq.tile([128, 1], f32, tag="s1")
                junkA = stq.tile([128, 4], f32, tag="junkA")
                nc.vector.tensor_scalar(
                    junkA[:], statb[:, i * 8 + 0:i * 8 + 8:2],
                    1.0, None, OP.mult, OP.add, accum_out=s1[:])
                s2 = stq.tile([128, 1], f32, tag="s2")
                junkB = stq.tile([128, 4], f32, tag="junkB")
                nc.vector.tensor_scalar(
                    junkB[:], statb[:, i * 8 + 1:i * 8 + 8:2],
                    1.0, None, OP.mult, OP.add, accum_out=s2[:])
                # mp = mu + C ; mp2 = mp^2 (ACT); m2r = S2/NTOT (ACT path)
                mp = stq.tile([128, 1], f32, tag="mp")
                nc.vector.tensor_scalar(
                    mp[:], s1[:], 1.0 / NTOT, C_SHIFT, OP.mult, OP.add)
                nc.vector.tensor_scalar(
                    negmu[:, out_idx + i:out_idx + i + 1], s1[:],
                    -1.0 / NTOT, None, OP.mult)
                mp2 = stq.tile([128, 1], f32, tag="mp2")
                nc.vector.scalar_tensor_tensor(
                    mp2[:], mp[:], 1.0, mp[:], OP.mult, OP.mult)
                m2r = stq.tile([128, 1], f32, tag="m2r")
                nc.vector.tensor_scalar(
                    m2r[:], s2[:], 1.0 / NTOT, None, OP.mult)
                var = stq.tile([128, 1], f32, tag="var")
                nc.vector.tensor_tensor(var[:], m2r[:], mp2[:], OP.subtract)
                sd = stq.tile([128, 1], f32, tag="sd")
                nc.scalar.activation(sd[:], var[:], AF.Sqrt, bias=cst[:, 1:2])
                nc.vector.reciprocal(rsv[:, out_idx + i:out_idx + i + 1],
                                     sd[:])
                nc.vector.tensor_tensor(
                    negmurs[:, out_idx + i:out_idx + i + 1],
                    negmu[:, out_idx + i:out_idx + i + 1],
                    rsv[:, out_idx + i:out_idx + i + 1], OP.mult)

        # broadcast fc bias [1,E] -> [128,E] on device (off critical path)
        psfc = psT.tile([128, E], f32, tag="psb")
        nc.tensor.matmul(psfc[:], onesrow[:], fcb1[:], start=True, stop=True)
        fcb = apool.tile([128, E], f32)
        nc.scalar.copy(fcb[:], psfc[:])

        co1sb = apool.tile([1, 24], f32)
        nc.gpsimd.dma_start(co1sb[:], cc1_out[:])
        psb1 = psT.tile([128, 24], f32, tag="psb")
        nc.tensor.matmul(psb1[:], onesrow[:], co1sb[:], start=True, stop=True)
        stat_math(psb1, 0, 3, 0)
        mctx.close()

        # ================= Phase B: LN + attention + out =================
        with ExitStack() as bctx:
            tpool = bctx.enter_context(tc.tile_pool(name="T", bufs=1))
            wpool = bctx.enter_context(tc.tile_pool(name="lnp", bufs=4))
            psB = bctx.enter_context(
                tc.tile_pool(name="psB", bufs=1, space="PSUM"))
            sb = bctx.enter_context(tc.tile_pool(name="sb", bufs=6))

            # --- feature-major LN-apply for q,k:
            #     TT = rs*G + lnbT + (-mu*rs)*lnwT    (G = YT*lnwT, pre-AR)
            TT = {}
            for p in range(2):
                for ei in range(3):
                    t1_ = wpool.tile([128, S], f32, tag="t1T")
                    nc.vector.scalar_tensor_tensor(
                        t1_[:], G[(p, ei)][:], rsv[:, p:p + 1],
                        lnbT[(p, ei)][:], OP.mult, OP.add)
                    tt_ = tpool.tile([128, S], f32, tag=f"TT{p}{ei}")
                    nc.vector.scalar_tensor_tensor(
                        tt_[:], lnwT[(p, ei)][:], negmurs[:, p:p + 1],
                        t1_[:], OP.mult, OP.add)
                    TT[(p, ei)] = tt_

            # --- token-major LN-apply for v (Gv precomputable pre-AR2)
            T2 = []
            for si, (s0, s1) in enumerate(SBLK):
                sP = s1 - s0
                yt = wpool.tile([sP, E], f32, tag="ytv")
                nc.sync.dma_start(
                    yt[:],
                    ybuf[2][s0 * E:s1 * E].rearrange("(a b) -> a b", b=E))
                dlw = wpool.tile([sP, E], bf16, tag="dlwv")
                nc.sync.dma_start(dlw[:], lnwv_d[s0:s1, :])
                lb = wpool.tile([sP, E], bf16, tag="lbv")
                nc.sync.dma_start(lb[:], lnbv_d[s0:s1, :])
                lw = wpool.tile([sP, E], f32, tag="lwv")
                nc.vector.tensor_scalar(lw[:], dlw[:], 1.0, None, OP.add)
                gv = wpool.tile([sP, E], f32, tag="gv")
                nc.vector.tensor_tensor(gv[:], yt[:], lw[:], OP.mult)
                t1_ = wpool.tile([sP, E], f32, tag="t1v")
                nc.vector.scalar_tensor_tensor(
                    t1_[:], gv[:], rsv[0:sP, 2:3], lb[:], OP.mult, OP.add)
                tt_ = tpool.tile([sP, E], f32, tag=f"T2{si}")
                nc.vector.scalar_tensor_tensor(
                    tt_[:], lw[:], negmurs[0:sP, 2:3], t1_[:],
                    OP.mult, OP.add)
                T2.append(tt_)

            # --- attention per head ---
            o_nat = [tpool.tile([s1 - s0, E], f32, tag=f"on{si}",
                                name=f"on{si}")
                     for si, (s0, s1) in enumerate(SBLK)]
            onacc = [tpool.tile([s1 - s0, 6], f32, name=f"onacc{si}")
                     for si, (s0, s1) in enumerate(SBLK)]
            for h in range(6):
                ei, r0 = (h * D) // 128, (h * D) % 128
                qT = TT[(0, ei)][r0:r0 + D, :]
                kT = TT[(1, ei)][r0:r0 + D, :]
                pexps = []
                rinvs = []
                for si, (s0, s1) in enumerate(SBLK):
                    sP = s1 - s0
                    sc = psB.tile([sP, S], f32, tag="sc", bufs=3)
                    nc.tensor.matmul(sc[:], qT[:, s0:s1], kT[:],
                                     start=True, stop=True)
                    pexp = sb.tile([sP, S], f32, tag="pexp")
                    rsum = sb.tile([sP, 1], f32, tag="rsum")
                    nc.scalar.activation(
                        pexp[:], sc[:], AF.Exp, scale=SCALE,
                        accum_out=rsum[:])
                    rinv = sb.tile([sP, 1], f32, tag="rinv")
                    nc.vector.reciprocal(rinv[:], rsum[:])
                    pexps.append(pexp)
                    rinvs.append(rinv)
                # transpose pexp -> peT tiles [tP, 197]
                peT = []
                for ti, (t0, t1) in enumerate(SBLK):
                    tP = t1 - t0
                    pat = psB.tile([tP, S], f32, tag="pat", bufs=2)
                    for si, (s0, s1) in enumerate(SBLK):
                        sP = s1 - s0
                        nc.tensor.transpose(
                            pat[:, s0:s1], pexps[si][:, t0:t1],
                            eyeq[0:sP, 0:sP])
                    at_ = sb.tile([tP, S], f32, tag="atT")
                    nc.scalar.copy(at_[:], pat[:])
                    peT.append(at_)
                # o = softmax@v + v: out[s,d] = rinv[s]*(pexp@v)[s,d] + v[s,d]
                for si, (s0, s1) in enumerate(SBLK):
                    sP = s1 - s0
                    ops_ = psB.tile([sP, D], f32, tag="ops", bufs=2)
                    for ti, (t0, t1) in enumerate(SBLK):
                        nc.tensor.matmul(
                            ops_[:],
                            peT[ti][:, s0:s1],
                            T2[ti][:, h * D:(h + 1) * D],
                            start=(ti == 0), stop=(ti == 1))
                    nc.vector.scalar_tensor_tensor(
                        o_nat[si][:, h * D:(h + 1) * D], ops_[:],
                        rinvs[si][:], T2[si][:, h * D:(h + 1) * D],
                        OP.mult, OP.add,
                        accum_out=onacc[si][:, h:h + 1])

            # --- token-local LayerNorm on o (affine folded into fc wts) ---
            oln = []
            for si, (s0, s1) in enumerate(SBLK):
                sP = s1 - s0
                on = o_nat[si]
                os1 = sb.tile([sP, 1], f32, tag="os1")
                junk1 = sb.tile([sP, 6], f32, tag="junk1")
                nc.vector.tensor_scalar(
                    junk1[:], onacc[si][:], 1.0, None, OP.mult, OP.add,
                    accum_out=os1[:])
                junk2 = sb.tile([sP, E], f32, tag="junkB2")
                os2 = sb.tile([sP, 1], f32, tag="os2")
                nc.scalar.activation(
                    junk2[:], on[:], AF.Square, accum_out=os2[:])
                nmuo = sb.tile([sP, 1], f32, tag="nmuo")
                nc.vector.tensor_scalar(
                    nmuo[:], os1[:], -1.0 / E, None, OP.mult)
                mu2o = sb.tile([sP, 1], f32, tag="mu2o")
                nc.scalar.activation(mu2o[:], nmuo[:], AF.Square)
                m2o = sb.tile([sP, 1], f32, tag="m2o")
                nc.vector.tensor_scalar(
                    m2o[:], os2[:], 1.0 / E, None, OP.mult)
                varo = sb.tile([sP, 1], f32, tag="varo")
                nc.vector.tensor_tensor(varo[:], m2o[:], mu2o[:], OP.subtract)
                sdo = sb.tile([sP, 1], f32, tag="sdo")
                nc.scalar.activation(
                    sdo[:], varo[:], AF.Sqrt, bias=cst[0:sP, 1:2])
                rso = sb.tile([sP, 1], f32, tag="rso")
                nc.vector.reciprocal(rso[:], sdo[:])
                z = sb.tile([sP, E], f32, tag="z")
                nc.vector.tensor_scalar(
                    z[:], on[:], nmuo[:], rso[:], OP.add, OP.mult)
                oln.append(z)

            # transpose oln -> [384, 197] feature-major for fc lhsT
            olnT = []
            for ei, (e0, e1) in enumerate(EBLK):
                pst = psB.tile([128, S], f32, tag="pat", bufs=2)
                for si, (s0, s1) in enumerate(SBLK):
                    sP = s1 - s0
                    nc.tensor.transpose(
                        pst[:, s0:s1], oln[si][:, e0:e1], eyeq[0:sP, 0:sP])
                ot = sb.tile([128, S], f32, tag=f"olnT{ei}")
                nc.scalar.copy(ot[:], pst[:])
                olnT.append(ot)

            for si, (s0, s1) in enumerate(SBLK):
                sP = s1 - s0
                fps = psB.tile([sP, E], f32, tag="fps")
                for ei in range(3):
                    nc.tensor.matmul(
                        fps[:], olnT[ei][:, s0:s1], fcw[ei][:],
                        start=(ei == 0), stop=(ei == 2))
                fin = sb.tile([sP, E], f32, tag="fin")
                nc.vector.scalar_tensor_tensor(
                    fin[:], fps[:], 1.0, fcb[0:sP, :], OP.mult, OP.add)
                nc.sync.dma_start(out_d[s0:s1, :], fin[:])

    nc.compile()
    return nc


def _fit_tables():
    """LS-fit relu(u - t) over t~|N(0,1)| with basis {1, min(t,tau_k)}.
    Returns (ugrid, coef [1+K, U])."""
    tq = np.linspace(0, 5.0, 20001)
    dtq = tq[1] - tq[0]
    dens = 2 * np.exp(-tq ** 2 / 2) / np.sqrt(2 * np.pi)
    Bm = np.stack([np.ones_like(tq)] + [np.minimum(tq, t) for t in TAUS])
    Wq = dens * dtq
    Gram = (Bm * Wq) @ Bm.T
    ugrid = np.linspace(0, 0.6, 3001)
    tgt = np.maximum(ugrid[:, None] - tq[None, :], 0.0)
    rhs = (Bm * Wq) @ tgt.T
    coef = np.linalg.solve(Gram, rhs)         # [1+K, U]
    return ugrid, coef


def _prep_inputs(inputs):
    """Build the 8 per-core input maps from full inputs."""
    x = np.ascontiguousarray(np.asarray(inputs["x"], dtype=np.float32))
    x2d = x.reshape(E, N)
    wq = np.asarray(inputs["wq"], dtype=np.float32)
    wk = np.asarray(inputs["wk"], dtype=np.float32)
    wv = np.asarray(inputs["wv"], dtype=np.float32)
    lnw = [np.asarray(inputs[k], dtype=np.float32).reshape(E, N)
           for k in ("qln_w", "kln_w", "vln_w")]
    lnb = [np.asarray(inputs[k], dtype=np.float32).reshape(E, N)
           for k in ("qln_b", "kln_b", "vln_b")]
    oln_w = np.asarray(inputs["oln_w"], dtype=np.float32)
    oln_b = np.asarray(inputs["oln_b"], dtype=np.float32)
    fc_w = np.asarray(inputs["fc_w"], dtype=np.float32)
    fc_b = np.asarray(inputs["fc_b"], dtype=np.float32)

    import ml_dtypes
    bf = ml_dtypes.bfloat16

    ugrid, coef = _fit_tables()

    def interp_coef(u):
        idx = np.clip(u, 0.0, 0.6) * (3000.0 / 0.6)
        i0 = np.floor(idx).astype(np.int64)
        fr = idx - i0
        i1 = np.minimum(i0 + 1, 3000)
        return coef[:, i0] * (1 - fr) + coef[:, i1] * fr   # [1+K, ...]

    onesrow = np.ones((1, 128), np.float32)
    onescol = np.ones((128, 2), np.float32)
    indqk = np.zeros((128, 2), np.float32)
    indqk[0:48, 0] = 1.0
    indqk[48:96, 1] = 1.0
    eyeq = np.zeros((128, 394), np.float32)
    ey = np.eye(S, dtype=np.float32)
    eyeq[:, 0:S] = ey[0:128]
    eyeq[0:69, 197:394] = ey[128:]
    # fold the out-LN affine into the fc weights:
    #   out = z @ (olnw*fcwt) + (olnb @ fcwt + fcb)
    fcwt = np.ascontiguousarray(fc_w.T * oln_w[:, None]).astype(np.float32)
    fcb1 = (oln_b @ fc_w.T + fc_b).astype(np.float32).reshape(1, E)
    cstcol = np.zeros((128, 2), np.float32)
    cstcol[:, 0] = C_SHIFT
    cstcol[:, 1] = EPS

    in_maps = []
    for c in range(NCORE):
        sl = slice(c * RPC, (c + 1) * RPC)
        w_core = np.concatenate([wq[sl], wk[sl], wv[sl]], axis=0)  # [144,384]
        u = np.abs(w_core)
        sw = np.sign(w_core)
        A = interp_coef(u)                       # [1+K, 144, 384]
        ones_m = np.ones_like(w_core)
        # weight matrices per basis: s, m_k, c_k, axb(-1)
        mats = [w_core - sw * A[0]]
        for k in range(K):
            mats.append(-A[1 + k])
        for k in range(K):
            mats.append(-sw * A[1 + k])
        mats.append(-ones_m)
        c0 = A[0].sum(axis=1)                    # [144]
        wbt = np.zeros((128, NB * 3 * CO), np.float32)
        for b in range(NB):
            mb = mats[b]                         # [144, 384]
            for t in range(3):
                wbt[:, (b * 3 + t) * CO:(b * 3 + t + 1) * CO] = (
                    mb[:, 128 * t:128 * t + 128].T)
        wbt = wbt.astype(bf)
        negc0 = np.zeros((128, 4), np.float32)
        negc0[0:96, 0] = -c0[0:96]
        negc0[0:48, 1] = -c0[96:144]
        negc0[:, 2] = negc0[:, 0] + C_SHIFT
        negc0[:, 3] = negc0[:, 1] + C_SHIFT

        # feature-major LN params for q,k: [E_loc, S] for this core's batch
        # (lnw sent as bf16 delta from 1.0 for precision at half the bytes)
        dlnwT = np.stack([
            np.ascontiguousarray(m[sl].reshape(S, E).T - 1.0)
            for m in lnw[0:2]])
        lnbT = np.stack([
            np.ascontiguousarray(m[sl].reshape(S, E).T) for m in lnb[0:2]])
        in_maps.append({
            "x2d": x2d.astype(bf),
            "wbt": wbt,
            "negc0": negc0,
            "onesrow": onesrow,
            "onescol": onescol,
            "indqk": indqk,
            "dlnwT": dlnwT.astype(bf),
            "lnbT": lnbT.astype(bf),
            "dlnwv": np.ascontiguousarray(
                lnw[2][sl].reshape(S, E) - 1.0).astype(bf),
            "lnbv": np.ascontiguousarray(
                lnb[2][sl].reshape(S, E)).astype(bf),
            "fcwt": fcwt,
            "fcb1": fcb1,
            "eyeq": eyeq,
            "cstcol": cstcol,
        })
    return in_maps


def get_program():
    global _PROGRAM
    if _PROGRAM is None:
        _PROGRAM = _build_program()
    return _PROGRAM


def kernel(**inputs):
    from concourse.bass_utils import run_bass_kernel_spmd
    nc = get_program()
    in_maps = _prep_inputs(inputs)
    res = run_bass_kernel_spmd(nc, in_maps, list(range(NCORE)))
    out = np.stack([res.results[c]["out"] for c in range(NCORE)])
    return out.astype(np.float32)


# revision 67
# speedup vs baseline: 1.4041x; 1.4041x over previous
"""Trainium2 Bass kernel for nn_Attention (AdderNet attention block).

Problem: B=8, S=197, E=384, H=6, D=64.
  x2d = x.reshape(E, B*S)                      # flat reshape, [384, 1576]
  per proj (q,k,v):  Y = -sum_ci |x2d[ci,n] - w[co,ci]|   (adder 1x1)
                     LN over ALL of [E,B,S] (elementwise affine params)
                     flat-reshape to [B,S,H,D] -> heads
  att = softmax(q k^T * scale) + I; o = att v; token-LN; fc.

Sharding: core c owns co-rows [48c, 48c+48) of each of the three adder
projections; those rows are exactly the post-LN data needed for batch
b=c of the attention, so attention + out-LN + fc are fully local per
core.  Cross-core exchange: two tiny AllReduces of LN statistics
(q+k stats early, v stats at phase-A end) overlapped with compute.

Adder projection via separable decomposition (instead of dense
elementwise abs-diff):
  |x-w| = |x| - sign(x)*w + relu(|w|-|x|)*(1 + sign(x)sign(w))
and relu(u-t) ~= a0(u) + sum_k ak(u) * min(t,tau_k)   (LS fit over
t ~ |N(0,1)|), giving
  Y = (W - Sw*A0)@s - sum_k AEk@m_k - sum_k (Sw*AOk)@c_k
      - 1@(axb+rb) - c0[co]
with device basis tiles s=sign(x), m_k=min(|x|,tau_k), c_k=s*m_k,
axb=bf16(|x|), rb=|x|-axb (residual, so colsum is fp32-accurate) in
bf16; weight tables host-precomputed in bf16; all 216 reductions are
PE matmuls.  Stats (sum, sum-of-squares) accumulate during PSUM
evacuation.  End-to-end rel err ~1.8e-3 (verified vs exact reference).
"""

import numpy as np
from contextlib import ExitStack

B, S, E = 8, 197, 384
H, D = 6, 64
N = B * S            # 1576
RPC = E // 8         # 48 rows per core per projection
NCORE = 8
NTOT = E * N         # 605184 elements per projection
C_SHIFT = 307.0      # conditioning shift for sum-of-squares (Y ~ -307)
EPS = 1e-5
SCALE = float((2.0 * D * (1.0 - 2.0 / np.pi)) ** (-0.5))
NCH = [(0, 512), (512, 1024), (1024, 1536), (1536, 1576)]
SBLK = [(0, 128), (128, 197)]     # token blocks of 197
EBLK = [(0, 128), (128, 256), (256, 384)]

TAUS = (0.05, 0.11, 0.2)
K = len(TAUS)
NB = 2 * K + 2       # basis: s, m_1..K, c_1..K, axb
CO = 3 * RPC         # 144 rows of stacked q/k/v weights per core
COG = [(0, 96), (96, 144)]        # co-groups: qk (M=96), v (M=48)

_PROGRAM = None


def _build_program(no_collective=False):
    import concourse.bass as bass
    import concourse.mybir as mybir
    from concourse import bacc, tile

    dt = mybir.dt
    f32 = dt.float32
    bf16 = dt.bfloat16
    AF = mybir.ActivationFunctionType
    OP = mybir.AluOpType

    nc = bacc.Bacc(num_devices=NCORE)

    # ---- I/O ----
    x2d_d = nc.dram_tensor("x2d", [E, N], bf16, kind="ExternalInput")
    wbt_d = nc.dram_tensor("wbt", [128, NB * 3 * CO], bf16,
                           kind="ExternalInput")
    negc0_d = nc.dram_tensor("negc0", [128, 4], f32, kind="ExternalInput")
    onesrow_d = nc.dram_tensor("onesrow", [1, 128], f32, kind="ExternalInput")
    onescol_d = nc.dram_tensor("onescol", [128, 2], f32, kind="ExternalInput")
    indqk_d = nc.dram_tensor("indqk", [128, 2], f32, kind="ExternalInput")
    lnwT_d = nc.dram_tensor("dlnwT", [2, E, S], bf16, kind="ExternalInput")
    lnbT_d = nc.dram_tensor("lnbT", [2, E, S], bf16, kind="ExternalInput")
    lnwv_d = nc.dram_tensor("dlnwv", [S, E], bf16, kind="ExternalInput")
    lnbv_d = nc.dram_tensor("lnbv", [S, E], bf16, kind="ExternalInput")
    fcwt_d = nc.dram_tensor("fcwt", [E, E], bf16, kind="ExternalInput")
    fcb_d = nc.dram_tensor("fcb1", [1, E], f32, kind="ExternalInput")
    eyeq_d = nc.dram_tensor("eyeq", [128, 128], bf16, kind="ExternalInput")
    cst_d = nc.dram_tensor("cstcol", [128, 2], f32, kind="ExternalInput")
    out_d = nc.dram_tensor("out", [S, E], f32, kind="ExternalOutput")

    # internal DRAM
    ybuf = [nc.dram_tensor(f"ybuf{p}", [RPC * N], f32) for p in range(3)]

    with ExitStack() as ctx:
        tc = ctx.enter_context(tile.TileContext(nc))
        const = ctx.enter_context(tc.tile_pool(name="const", bufs=1))

        # x tiles FIRST: everything downstream gates on them.  wbt arrives
        # in per-basis slices so the first matmul stack can start early.
        xp0 = ctx.enter_context(tc.tile_pool(name="xp0", bufs=1))
        xts = []
        wbt = const.tile([128, NB * 3 * CO], bf16)
        for t in range(3):
            xt = xp0.tile([128, N], bf16, name=f"xt{t}")
            nc.sync.dma_start(xt[:], x2d_d[128 * t:128 * t + 128, :])
            xts.append(xt)
            if t == 0:
                nc.sync.dma_start(wbt[:, 0:3 * CO], wbt_d[:, 0:3 * CO])
        for b in range(1, NB):
            nc.sync.dma_start(wbt[:, b * 3 * CO:(b + 1) * 3 * CO],
                              wbt_d[:, b * 3 * CO:(b + 1) * 3 * CO])
        negc0 = const.tile([128, 4], f32)
        nc.sync.dma_start(negc0[:], negc0_d[:])
        onesrow = const.tile([1, 128], f32)
        nc.sync.dma_start(onesrow[:], onesrow_d[:])
        onescol = const.tile([128, 2], f32)
        nc.sync.dma_start(onescol[:], onescol_d[:])
        indqk = const.tile([128, 2], f32)
        nc.sync.dma_start(indqk[:], indqk_d[:])
        cst = const.tile([128, 2], f32)
        nc.sync.dma_start(cst[:], cst_d[:])
        lnwT = {}
        lnbT = {}
        for p in range(2):
            for ei, (e0, e1) in enumerate(EBLK):
                dl = const.tile([128, S], bf16, name=f"dlnwT{p}{ei}")
                nc.sync.dma_start(dl[:], lnwT_d[p, e0:e1, :])
                lw = const.tile([128, S], f32, name=f"lnwT{p}{ei}")
                nc.vector.tensor_scalar(lw[:], dl[:], 1.0, None, OP.add)
                lnwT[(p, ei)] = lw
                lb = const.tile([128, S], bf16, name=f"lnbT{p}{ei}")
                nc.sync.dma_start(lb[:], lnbT_d[p, e0:e1, :])
                lnbT[(p, ei)] = lb
        eyeb = const.tile([128, 128], bf16)
        nc.sync.dma_start(eyeb[:], eyeq_d[:])
        eyef = const.tile([128, 128], f32)
        nc.scalar.copy(eyef[:], eyeb[:])
        fcb1 = const.tile([1, E], f32)
        nc.sync.dma_start(fcb1[:], fcb_d[:])
        fcw = []
        for ei, (e0, e1) in enumerate(EBLK):
            fw = const.tile([128, E], bf16, tag=f"fw{ei}", name=f"fw{ei}")
            nc.sync.dma_start(fw[:], fcwt_d[e0:e1, :])
            fcw.append(fw)

        # stat result tiles
        rsv = const.tile([128, 3], f32)      # 1/sqrt(var+eps) per proj
        negmu = const.tile([128, 3], f32)    # -mu per proj
        negmurs = const.tile([128, 3], f32)  # -mu*rs per proj

        ccdram = ctx.enter_context(
            tc.tile_pool(name="ccdram", bufs=1, space="DRAM"))
        cc1_in = ccdram.tile([1, 24], f32, name="cc1_in")
        cc1_out = ccdram.tile([1, 24], f32, name="cc1_out")

        cc0_in = ccdram.tile([1, 4], f32, name="cc0_in")
        cc0_out = ccdram.tile([1, 4], f32, name="cc0_out")
        # dummy first collective: pays the comms-channel init/rendezvous
        # cost off the critical path so the real stats AllReduce is fast.
        # Also warm the ACT table that holds Sqrt.
        dum = const.tile([1, 4], f32)
        nc.vector.memset(dum[:], 1.0)
        nc.gpsimd.dma_start(cc0_in[:], dum[:])
        if no_collective:
            nc.gpsimd.dma_start(cc0_out[:], cc0_in[:])
        else:
            nc.gpsimd.collective_compute(
                "AllReduce", mybir.AluOpType.add,
                replica_groups=[list(range(NCORE))],
                ins=[cc0_in.opt()], outs=[cc0_out.opt()])
        warm = const.tile([1, 4], f32)
        nc.scalar.activation(warm[:], dum[:], AF.Sqrt)

        # ================= Phase A: separable adder projections ==========
        apool = ctx.enter_context(tc.tile_pool(name="apool", bufs=1))
        mctx = ExitStack()   # closed before phase B to release PSUM banks
        psT = mctx.enter_context(tc.tile_pool(name="psT", bufs=1,
                                              space="PSUM"))
        with ExitStack() as actx:
            bp = actx.enter_context(tc.tile_pool(name="bp", bufs=1))
            evp = actx.enter_context(tc.tile_pool(name="evp", bufs=3))

            # basis tiles bs[b][t]; b: 0=s, 1..K=m_k, K+1..2K=c_k, 2K+1=axb
            bs = [[bp.tile([128, N], bf16, name=f"bs{b}_{t}")
                   for t in range(3)] for b in range(NB)]

            for t in range(3):
                xt = xts[t]
                nc.scalar.activation(bs[0][t][:], xt[:], AF.Sign)
                nc.scalar.activation(bs[2 * K + 1][t][:], xt[:], AF.Abs)
                for k in range(K):
                    nc.vector.tensor_scalar(
                        bs[1 + k][t][:], bs[2 * K + 1][t][:], 1.0, TAUS[k],
                        OP.mult, OP.min)
                    nc.vector.tensor_tensor(
                        bs[1 + K + k][t][:], bs[1 + k][t][:], bs[0][t][:],
                        OP.mult)

            # stats accumulators (per-partition, per chunk: sum, sumsq)
            ssqk = apool.tile([128, 8], f32)
            ssv = apool.tile([128, 8], f32)

            # main matmul stacks, qk group first then v
            psQK = actx.enter_context(
                tc.tile_pool(name="psQK", bufs=3, space="PSUM"))
            psV = actx.enter_context(
                tc.tile_pool(name="psV", bufs=2, space="PSUM"))
            for gi, (co0, co1) in enumerate(COG):
                M = co1 - co0
                pool_g = psQK if gi == 0 else psV
                for ci_, (a, b_) in enumerate(NCH):
                    ps = pool_g.tile([M, b_ - a], f32, tag="ps")
                    i = 0
                    for b in range(NB):
                        for t in range(3):
                            off = (b * 3 + t) * CO + co0
                            nc.tensor.matmul(
                                ps[:], wbt[:, off:off + M],
                                bs[b][t][:, a:b_],
                                start=(i == 0), stop=(i == NB * 3 - 1))
                            i += 1
                    # evac + stats on DVE: ev = ps + negc0 (accum sum);
                    # t3 = ps + negc0 + C ; sq = t3*t3 (accum sumsq)
                    ev = evp.tile([M, b_ - a], f32, tag="evac")
                    ss = ssqk if gi == 0 else ssv
                    nc.vector.tensor_scalar(
                        ev[:], ps[:], negc0[0:M, gi:gi + 1], None, OP.add,
                        OP.add, accum_out=ss[0:M, 2 * ci_:2 * ci_ + 1])
                    t3 = evp.tile([M, b_ - a], f32, tag="t3ev")
                    nc.vector.tensor_scalar(
                        t3[:], ps[:], negc0[0:M, 2 + gi:3 + gi], None,
                        OP.add)
                    junk = evp.tile([M, b_ - a], f32, tag="junkev")
                    nc.vector.scalar_tensor_tensor(
                        junk[:], t3[:], 1.0, t3[:], OP.mult, OP.mult,
                        accum_out=ss[0:M, 2 * ci_ + 1:2 * ci_ + 2])
                    if gi == 0:
                        for p in range(2):
                            nc.gpsimd.dma_start(
                                ybuf[p][:].rearrange(
                                    "(r n) -> r n", n=N)[:, a:b_],
                                ev[48 * p:48 * p + 48, :])
                    else:
                        nc.gpsimd.dma_start(
                            ybuf[2][:].rearrange(
                                "(r n) -> r n", n=N)[:, a:b_],
                            ev[0:48, :])

                # per-group stats partition-reduction (PE + copy)
                if gi == 0:
                    prqk = psT.tile([2, 8], f32, tag="pr")
                    nc.tensor.matmul(prqk[:], indqk[0:96, :], ssqk[0:96, :],
                                     start=True, stop=True)
                    prqk_sb = apool.tile([2, 8], f32)
                    nc.scalar.copy(prqk_sb[:], prqk[:])
                    nc.gpsimd.dma_start(cc1_in[0:1, 0:8], prqk_sb[0:1, :])
                    nc.gpsimd.dma_start(cc1_in[0:1, 8:16], prqk_sb[1:2, :])
                else:
                    prv = psT.tile([1, 8], f32, tag="pr")
                    nc.tensor.matmul(prv[:], onescol[0:48, 0:1], ssv[0:48, :],
                                     start=True, stop=True)
                    prv_sb = apool.tile([1, 8], f32)
                    nc.scalar.copy(prv_sb[:], prv[:])
                    nc.gpsimd.dma_start(cc1_in[0:1, 16:24], prv_sb[:])
                    if no_collective:
                        nc.gpsimd.dma_start(cc1_out[:], cc1_in[:])
                    else:
                        nc.gpsimd.collective_compute(
                            "AllReduce", mybir.AluOpType.add,
                            replica_groups=[list(range(NCORE))],
                            ins=[cc1_in.opt()], outs=[cc1_out.opt()])

        # ============ pre-LN feature-major transposes for q, k ============
        # (overlaps the v matmul stacks and the q/k AllReduce)
        # G = YT * lnwT precomputed so post-AllReduce LN is 2 ops per tile.
        G = {}
        YT = {}
        ytp = mctx.enter_context(tc.tile_pool(name="ytp", bufs=4))
        for p in range(2):
            for ei, (e0, e1) in enumerate(EBLK):
                pst = psT.tile([128, S], f32, tag="pst")
                for si, (s0, s1) in enumerate(SBLK):
                    sP = s1 - s0
                    yt = ytp.tile([sP, 128], f32, tag="ytqk")
                    nc.sync.dma_start(
                        yt[:],
                        ybuf[p][:].rearrange(
                            "(s e) -> s e", e=E)[s0:s1, e0:e1])
                    nc.tensor.transpose(
                        pst[:, s0:s1], yt[:], eyef[0:sP, 0:sP])
                ytt = apool.tile([128, S], f32, name=f"YT{p}{ei}")
                nc.scalar.copy(ytt[:], pst[:])
                YT[(p, ei)] = ytt
                g_ = apool.tile([128, S], f32, name=f"G{p}{ei}")
                nc.vector.tensor_tensor(g_[:], ytt[:], lnwT[(p, ei)][:],
                                        OP.mult)
                G[(p, ei)] = g_

        # ================= stats scalar math =================
        stq = mctx.enter_context(tc.tile_pool(name="stq", bufs=2))

        def stat_math(statb, pcol, ncols, out_idx):
            """statb: [128, ncols*8] bcast stats; for each local proj i,
            columns i*8 + (chunk*2 + j); compute rs, negmu into
            rsv/negmu[:, out_idx+i]."""
            for i in range(ncols):
                s1 = stq.tile([128, 1], f32, tag="s1")
                junkA = stq.tile([128, 4], f32, tag="junkA")
                nc.vector.tensor_scalar(
                    junkA[:], statb[:, i * 8 + 0:i * 8 + 8:2],
                    1.0, None, OP.mult, OP.add, accum_out=s1[:])
                s2 = stq.tile([128, 1], f32, tag="s2")
                junkB = stq.tile([128, 4], f32, tag="junkB")
                nc.vector.tensor_scalar(
                    junkB[:], statb[:, i * 8 + 1:i * 8 + 8:2],
                    1.0, None, OP.mult, OP.add, accum_out=s2[:])
                # mp = mu + C ; mp2 = mp^2 (ACT); m2r = S2/NTOT (ACT path)
                mp = stq.tile([128, 1], f32, tag="mp")
                nc.vector.tensor_scalar(
                    mp[:], s1[:], 1.0 / NTOT, C_SHIFT, OP.mult, OP.add)
                nc.vector.tensor_scalar(
                    negmu[:, out_idx + i:out_idx + i + 1], s1[:],
                    -1.0 / NTOT, None, OP.mult)
                mp2 = stq.tile([128, 1], f32, tag="mp2")
                nc.vector.scalar_tensor_tensor(
                    mp2[:], mp[:], 1.0, mp[:], OP.mult, OP.mult)
                m2r = stq.tile([128, 1], f32, tag="m2r")
                nc.vector.tensor_scalar(
                    m2r[:], s2[:], 1.0 / NTOT, None, OP.mult)
                var = stq.tile([128, 1], f32, tag="var")
                nc.vector.tensor_tensor(var[:], m2r[:], mp2[:], OP.subtract)
                sd = stq.tile([128, 1], f32, tag="sd")
                nc.scalar.activation(sd[:], var[:], AF.Sqrt, bias=cst[:, 1:2])
                nc.vector.reciprocal(rsv[:, out_idx + i:out_idx + i + 1],
                                     sd[:])
                nc.vector.tensor_tensor(
                    negmurs[:, out_idx + i:out_idx + i + 1],
                    negmu[:, out_idx + i:out_idx + i + 1],
                    rsv[:, out_idx + i:out_idx + i + 1], OP.mult)

        # broadcast fc bias [1,E] -> [128,E] on device (off critical path)
        psfc = psT.tile([128, E], f32, tag="psb")
        nc.tensor.matmul(psfc[:], onesrow[:], fcb1[:], start=True, stop=True)
        fcb = apool.tile([128, E], f32)
        nc.scalar.copy(fcb[:], psfc[:])

        co1sb = apool.tile([1, 24], f32)
        nc.gpsimd.dma_start(co1sb[:], cc1_out[:])
        psb1 = psT.tile([128, 24], f32, tag="psb")
        nc.tensor.matmul(psb1[:], onesrow[:], co1sb[:], start=True, stop=True)
        stat_math(psb1, 0, 3, 0)
        mctx.close()

        # ================= Phase B: LN + attention + out =================
        with ExitStack() as bctx:
            tpool = bctx.enter_context(tc.tile_pool(name="T", bufs=1))
            wpool = bctx.enter_context(tc.tile_pool(name="lnp", bufs=4))
            psB = bctx.enter_context(
                tc.tile_pool(name="psB", bufs=1, space="PSUM"))
            sb = bctx.enter_context(tc.tile_pool(name="sb", bufs=6))

            # --- feature-major LN-apply for q,k:
            #     TT = rs*G + lnbT + (-mu*rs)*lnwT    (G = YT*lnwT, pre-AR)
            TT = {}
            for p in range(2):
                for ei in range(3):
                    t1_ = wpool.tile([128, S], f32, tag="t1T")
                    nc.vector.scalar_tensor_tensor(
                        t1_[:], G[(p, ei)][:], rsv[:, p:p + 1],
                        lnbT[(p, ei)][:], OP.mult, OP.add)
                    tt_ = tpool.tile([128, S], bf16, tag=f"TT{p}{ei}")
                    nc.vector.scalar_tensor_tensor(
                        tt_[:], lnwT[(p, ei)][:], negmurs[:, p:p + 1],
                        t1_[:], OP.mult, OP.add)
                    TT[(p, ei)] = tt_

            # --- token-major LN-apply for v (Gv precomputable pre-AR2)
            T2 = []
            for si, (s0, s1) in enumerate(SBLK):
                sP = s1 - s0
                yt = wpool.tile([sP, E], f32, tag="ytv")
                nc.sync.dma_start(
                    yt[:],
                    ybuf[2][s0 * E:s1 * E].rearrange("(a b) -> a b", b=E))
                dlw = wpool.tile([sP, E], bf16, tag="dlwv")
                nc.sync.dma_start(dlw[:], lnwv_d[s0:s1, :])
                lb = wpool.tile([sP, E], bf16, tag="lbv")
                nc.sync.dma_start(lb[:], lnbv_d[s0:s1, :])
                lw = wpool.tile([sP, E], f32, tag="lwv")
                nc.vector.tensor_scalar(lw[:], dlw[:], 1.0, None, OP.add)
                gv = wpool.tile([sP, E], f32, tag="gv")
                nc.vector.tensor_tensor(gv[:], yt[:], lw[:], OP.mult)
                t1_ = wpool.tile([sP, E], f32, tag="t1v")
                nc.vector.scalar_tensor_tensor(
                    t1_[:], gv[:], rsv[0:sP, 2:3], lb[:], OP.mult, OP.add)
                tt_ = tpool.tile([sP, E], bf16, tag=f"T2{si}")
                nc.vector.scalar_tensor_tensor(
                    tt_[:], lw[:], negmurs[0:sP, 2:3], t1_[:],
                    OP.mult, OP.add)
                T2.append(tt_)

            # --- attention per head ---
            o_nat = [tpool.tile([s1 - s0, E], f32, tag=f"on{si}",
                                name=f"on{si}")
                     for si, (s0, s1) in enumerate(SBLK)]
            onacc = [tpool.tile([s1 - s0, 6], f32, name=f"onacc{si}")
                     for si, (s0, s1) in enumerate(SBLK)]
            for h in range(6):
                ei, r0 = (h * D) // 128, (h * D) % 128
                qT = TT[(0, ei)][r0:r0 + D, :]
                kT = TT[(1, ei)][r0:r0 + D, :]
                pexps = []
                rinvs = []
                for si, (s0, s1) in enumerate(SBLK):
                    sP = s1 - s0
                    sc = psB.tile([sP, S], f32, tag="sc", bufs=3)
                    nc.tensor.matmul(sc[:], qT[:, s0:s1], kT[:],
                                     start=True, stop=True)
                    pexp = sb.tile([sP, S], bf16, tag="pexp")
                    rsum = sb.tile([sP, 1], f32, tag="rsum")
                    nc.scalar.activation(
                        pexp[:], sc[:], AF.Exp, scale=SCALE,
                        accum_out=rsum[:])
                    rinv = sb.tile([sP, 1], f32, tag="rinv")
                    nc.vector.reciprocal(rinv[:], rsum[:])
                    pexps.append(pexp)
                    rinvs.append(rinv)
                # transpose pexp -> peT tiles [tP, 197]
                peT = []
                for ti, (t0, t1) in enumerate(SBLK):
                    tP = t1 - t0
                    pat = psB.tile([tP, S], bf16, tag="pat", bufs=2)
                    for si, (s0, s1) in enumerate(SBLK):
                        sP = s1 - s0
                        nc.tensor.transpose(
                            pat[:, s0:s1], pexps[si][:, t0:t1],
                            eyeb[0:sP, 0:sP])
                    at_ = sb.tile([tP, S], bf16, tag="atT")
                    nc.scalar.copy(at_[:], pat[:])
                    peT.append(at_)
                # o = softmax@v + v: out[s,d] = rinv[s]*(pexp@v)[s,d] + v[s,d]
                for si, (s0, s1) in enumerate(SBLK):
                    sP = s1 - s0
                    ops_ = psB.tile([sP, D], f32, tag="ops", bufs=2)
                    for ti, (t0, t1) in enumerate(SBLK):
                        nc.tensor.matmul(
                            ops_[:],
                            peT[ti][:, s0:s1],
                            T2[ti][:, h * D:(h + 1) * D],
                            start=(ti == 0), stop=(ti == 1))
                    nc.vector.scalar_tensor_tensor(
                        o_nat[si][:, h * D:(h + 1) * D], ops_[:],
                        rinvs[si][:], T2[si][:, h * D:(h + 1) * D],
                        OP.mult, OP.add,
                        accum_out=onacc[si][:, h:h + 1])

            # --- token-local LayerNorm on o (affine folded into fc wts) ---
            oln = []
            for si, (s0, s1) in enumerate(SBLK):
                sP = s1 - s0
                on = o_nat[si]
                os1 = sb.tile([sP, 1], f32, tag="os1")
                junk1 = sb.tile([sP, 6], f32, tag="junk1")
                nc.vector.tensor_scalar(
                    junk1[:], onacc[si][:], 1.0, None, OP.mult, OP.add,
                    accum_out=os1[:])
                junk2 = sb.tile([sP, E], f32, tag="junkB2")
                os2 = sb.tile([sP, 1], f32, tag="os2")
                nc.scalar.activation(
                    junk2[:], on[:], AF.Square, accum_out=os2[:])
                nmuo = sb.tile([sP, 1], f32, tag="nmuo")
                nc.vector.tensor_scalar(
                    nmuo[:], os1[:], -1.0 / E, None, OP.mult)
                mu2o = sb.tile([sP, 1], f32, tag="mu2o")
                nc.scalar.activation(mu2o[:], nmuo[:], AF.Square)
                m2o = sb.tile([sP, 1], f32, tag="m2o")
                nc.vector.tensor_scalar(
                    m2o[:], os2[:], 1.0 / E, None, OP.mult)
                varo = sb.tile([sP, 1], f32, tag="varo")
                nc.vector.tensor_tensor(varo[:], m2o[:], mu2o[:], OP.subtract)
                sdo = sb.tile([sP, 1], f32, tag="sdo")
                nc.scalar.activation(
                    sdo[:], varo[:], AF.Sqrt, bias=cst[0:sP, 1:2])
                rso = sb.tile([sP, 1], f32, tag="rso")
                nc.vector.reciprocal(rso[:], sdo[:])
                z = sb.tile([sP, E], bf16, tag="z")
                nc.vector.tensor_scalar(
                    z[:], on[:], nmuo[:], rso[:], OP.add, OP.mult)
                oln.append(z)

            # transpose oln -> [384, 197] feature-major for fc lhsT
            olnT = []
            for ei, (e0, e1) in enumerate(EBLK):
                pst = psB.tile([128, S], bf16, tag="pat", bufs=2)
                for si, (s0, s1) in enumerate(SBLK):
                    sP = s1 - s0
                    nc.tensor.transpose(
                        pst[:, s0:s1], oln[si][:, e0:e1], eyeb[0:sP, 0:sP])
                ot = sb.tile([128, S], bf16, tag=f"olnT{ei}")
                nc.scalar.copy(ot[:], pst[:])
                olnT.append(ot)

            for si, (s0, s1) in enumerate(SBLK):
                sP = s1 - s0
                fps = psB.tile([sP, E], f32, tag="fps")
                for ei in range(3):
                    nc.tensor.matmul(
                        fps[:], olnT[ei][:, s0:s1], fcw[ei][:],
                        start=(ei == 0), stop=(ei == 2))
                fin = sb.tile([sP, E], f32, tag="fin")
                nc.vector.scalar_tensor_tensor(
                    fin[:], fps[:], 1.0, fcb[0:sP, :], OP.mult, OP.add)
                nc.sync.dma_start(out_d[s0:s1, :], fin[:])

    nc.compile()
    return nc


def _fit_tables():
    """LS-fit relu(u - t) over t~|N(0,1)| with basis {1, min(t,tau_k)}.
    Returns (ugrid, coef [1+K, U])."""
    tq = np.linspace(0, 5.0, 20001)
    dtq = tq[1] - tq[0]
    dens = 2 * np.exp(-tq ** 2 / 2) / np.sqrt(2 * np.pi)
    Bm = np.stack([np.ones_like(tq)] + [np.minimum(tq, t) for t in TAUS])
    Wq = dens * dtq
    Gram = (Bm * Wq) @ Bm.T
    ugrid = np.linspace(0, 0.6, 3001)
    tgt = np.maximum(ugrid[:, None] - tq[None, :], 0.0)
    rhs = (Bm * Wq) @ tgt.T
    coef = np.linalg.solve(Gram, rhs)         # [1+K, U]
    return ugrid, coef


def _prep_inputs(inputs):
    """Build the 8 per-core input maps from full inputs."""
    x = np.ascontiguousarray(np.asarray(inputs["x"], dtype=np.float32))
    x2d = x.reshape(E, N)
    wq = np.asarray(inputs["wq"], dtype=np.float32)
    wk = np.asarray(inputs["wk"], dtype=np.float32)
    wv = np.asarray(inputs["wv"], dtype=np.float32)
    lnw = [np.asarray(inputs[k], dtype=np.float32).reshape(E, N)
           for k in ("qln_w", "kln_w", "vln_w")]
    lnb = [np.asarray(inputs[k], dtype=np.float32).reshape(E, N)
           for k in ("qln_b", "kln_b", "vln_b")]
    oln_w = np.asarray(inputs["oln_w"], dtype=np.float32)
    oln_b = np.asarray(inputs["oln_b"], dtype=np.float32)
    fc_w = np.asarray(inputs["fc_w"], dtype=np.float32)
    fc_b = np.asarray(inputs["fc_b"], dtype=np.float32)

    import ml_dtypes
    bf = ml_dtypes.bfloat16

    ugrid, coef = _fit_tables()

    def interp_coef(u):
        idx = np.clip(u, 0.0, 0.6) * (3000.0 / 0.6)
        i0 = np.floor(idx).astype(np.int64)
        fr = idx - i0
        i1 = np.minimum(i0 + 1, 3000)
        return coef[:, i0] * (1 - fr) + coef[:, i1] * fr   # [1+K, ...]

    onesrow = np.ones((1, 128), np.float32)
    onescol = np.ones((128, 2), np.float32)
    indqk = np.zeros((128, 2), np.float32)
    indqk[0:48, 0] = 1.0
    indqk[48:96, 1] = 1.0
    eyeq = np.eye(128, dtype=np.float32)
    # fold the out-LN affine into the fc weights:
    #   out = z @ (olnw*fcwt) + (olnb @ fcwt + fcb)
    fcwt = np.ascontiguousarray(fc_w.T * oln_w[:, None]).astype(np.float32)
    fcb1 = (oln_b @ fc_w.T + fc_b).astype(np.float32).reshape(1, E)
    cstcol = np.zeros((128, 2), np.float32)
    cstcol[:, 0] = C_SHIFT
    cstcol[:, 1] = EPS

    in_maps = []
    for c in range(NCORE):
        sl = slice(c * RPC, (c + 1) * RPC)
        w_core = np.concatenate([wq[sl], wk[sl], wv[sl]], axis=0)  # [144,384]
        u = np.abs(w_core)
        sw = np.sign(w_core)
        A = interp_coef(u)                       # [1+K, 144, 384]
        ones_m = np.ones_like(w_core)
        # weight matrices per basis: s, m_k, c_k, axb(-1)
        mats = [w_core - sw * A[0]]
        for k in range(K):
            mats.append(-A[1 + k])
        for k in range(K):
            mats.append(-sw * A[1 + k])
        mats.append(-ones_m)
        c0 = A[0].sum(axis=1)                    # [144]
        wbt = np.zeros((128, NB * 3 * CO), np.float32)
        for b in range(NB):
            mb = mats[b]                         # [144, 384]
            for t in range(3):
                wbt[:, (b * 3 + t) * CO:(b * 3 + t + 1) * CO] = (
                    mb[:, 128 * t:128 * t + 128].T)
        wbt = wbt.astype(bf)
        negc0 = np.zeros((128, 4), np.float32)
        negc0[0:96, 0] = -c0[0:96]
        negc0[0:48, 1] = -c0[96:144]
        negc0[:, 2] = negc0[:, 0] + C_SHIFT
        negc0[:, 3] = negc0[:, 1] + C_SHIFT

        # feature-major LN params for q,k: [E_loc, S] for this core's batch
        # (lnw sent as bf16 delta from 1.0 for precision at half the bytes)
        dlnwT = np.stack([
            np.ascontiguousarray(m[sl].reshape(S, E).T - 1.0)
            for m in lnw[0:2]])
        lnbT = np.stack([
            np.ascontiguousarray(m[sl].reshape(S, E).T) for m in lnb[0:2]])
        in_maps.append({
            "x2d": x2d.astype(bf),
            "wbt": wbt,
            "negc0": negc0,
            "onesrow": onesrow,
            "onescol": onescol,
            "indqk": indqk,
            "dlnwT": dlnwT.astype(bf),
            "lnbT": lnbT.astype(bf),
            "dlnwv": np.ascontiguousarray(
                lnw[2][sl].reshape(S, E) - 1.0).astype(bf),
            "lnbv": np.ascontiguousarray(
                lnb[2][sl].reshape(S, E)).astype(bf),
            "fcwt": fcwt.astype(bf),
            "fcb1": fcb1,
            "eyeq": eyeq.astype(bf),
            "cstcol": cstcol,
        })
    return in_maps


def get_program():
    global _PROGRAM
    if _PROGRAM is None:
        _PROGRAM = _build_program()
    return _PROGRAM


def kernel(**inputs):
    from concourse.bass_utils import run_bass_kernel_spmd
    nc = get_program()
    in_maps = _prep_inputs(inputs)
    res = run_bass_kernel_spmd(nc, in_maps, list(range(NCORE)))
    out = np.stack([res.results[c]["out"] for c in range(NCORE)])
    return out.astype(np.float32)
